# revision 22
# baseline (speedup 1.0000x reference)
"""Trainium2 Bass kernel for nn_MetaPN (hypernetwork MLP), v3.

v3 over v2 (~357us -> ~333us min, PE busy ~93%, PE gaps ~2-4us):
  - all layer-3 scale ops on VectorE (the ScalarE per-d path measured ~2x
    its modeled cost and was the L3 bottleneck)
  - CONSTB laid out so one small first DMA chunk unblocks the first 4
    matmuls; v2p/pbt/v3 tiles for the first groups prefetched explicitly
  - PE warm-up matmuls on a memset tile while head DMAs are in flight
    (HAM clock gate reaches 8/8 before real work)
  - layer-3 bias matmuls + x2 transposes interleaved into the first
    h3-generation block to fill the L2->L3 pipeline boundary
  - CONSTB rows region DMAs only partition 0 (was moving 704KB of zeros
    in the head-critical window), W3b/b3w bias segments unpadded (N=64
    bias matmuls), per-bank-pair output DMA, LAGM=3 drain
  Run-to-run variance is +/-8-20% from P0 power-state downclock with all
  8 cores at high PE utilization; compare minima across runs.

v2 notes:

Math (per sample b):
  x1 = prelu(coods @ w1 + bb1),  w1 = (pe @ W1w.T + b1w).reshape(2, D)
  x2 = prelu(sum_d x1[d] * w2[d, :] + bb2),  w2 = (pe @ W2w.T + b2w).reshape(D, D)
  x3 = sum_d x2[d] * w3[d, :] + bb3,         w3 = (pe @ W3w.T + b3w).reshape(D, DT)

v2 strategy (pure data parallel over batch, 8 cores x 512 samples):
  Layer 2 is computed TRANSPOSED with the per-sample bilinear form folded
  into one long PSUM accumulation:
      x2T[e, b] = sum_{k, d} V2[d, e, k] * (x1[b, d] * pe[b, k])
  - moving operand Z_k[d, b] = x1T[d, b] * peBC_k[:, b], where peBC_k is
    pe[:, k] replicated across partitions HOST-SIDE (DMA'd from DRAM);
    Z is built by single DVE tensor_tensor ops (SBUF-only, bf16).
  - stationary operands are host-permuted V2 slices [128 d x 128 e],
    4 matmuls (dc x eh) of N=512 per k accumulate into 2 PSUM banks.
  This removes the entire per-d scaling stage and all identity-matmul
  accumulation for layer 2 (the v1 bottleneck: ScalarE 80% busy).
  Layer 1 is computed directly in transposed form (x1T) with the coods
  combine using host-replicated coods rows.
  Layer 3 keeps weight-gen + scale + identity-accumulate, but the scale
  stage is one broadcast-AP tensor_tensor per (q, bt) on DVE (+ ScalarE
  per-d ops for one bt) instead of 8 tiny ops.
"""

import os

import numpy as np

import concourse.bass as bass
from concourse import bacc
import concourse.mybir as mybir
from concourse.tile import TileContext
from concourse.bass_utils import run_bass_kernel_spmd

D = 256
DT = 64
B = 4096
NCORES = 8
BP = B // NCORES          # samples per core = 512
NBT = BP // 128           # batch tiles per core = 4
KC = 2                    # contraction chunks of 128 over k (=D=256)
KG = 8                    # k's per DMA group for layer 2
NG = D // KG              # 32 k-groups
NQ = D // 8               # layer-3 d-groups (8 d's each)
ALPHA = 0.25              # PReLU alpha (nn.PReLU default from setup_inputs)

F32 = mybir.dt.float32
BF16 = mybir.dt.bfloat16

# CONSTB (bf16) column offsets -- L1-critical columns first so the first
# (small) DMA chunk unblocks the first 4 matmuls as early as possible.
O_W1A = 0                 # h1aT stationaries (dch,kc) 4x128
O_PETKC = 512             # peT*c0 / peT*c1 kc-split [128, 2*2*512]
O_PETK = 2560             # peT kc-split     [128, 2*512]
O_CHUNK1 = 1536           # end of the first DMA chunk (W1A + petkc ci=0)
O_W1B = 3584              # h1bT stationaries (dch,kc) 4x128
O_W1BB = 4096             # bb1T stationaries (dch,kc) 4x128
O_ROWS = 4608             # partition-0 rows
O_R_B1B = O_ROWS
O_R_B1W = O_ROWS + 256    # b1w rows: half-a [256], half-b [256]
O_R_C0 = O_ROWS + 768     # coods[:,0] row [512]
O_R_C1 = O_ROWS + 1280    # coods[:,1] row [512]
O_R_ONES = O_ROWS + 1792  # ones row [512]
O_R_B2B = O_ROWS + 2304
O_R_B3B = O_ROWS + 2560
O_SPLIT = O_ROWS + 2816   # 7424: end of the early-DMA region
O_W2B = O_SPLIT           # bb2T stationaries (eh,kc)  4x128
O_B2 = O_SPLIT + 512      # B2 = b2w.reshape  (eh,dc)  4x128
O_ID = O_SPLIT + 1024     # identity [128, 128]
O_W3B = O_SPLIT + 1152    # W3b.T kc-split [128, 2*64]
O_B3 = O_SPLIT + 1280     # B3 = b3w.reshape(D,DT) kc-split [128, 2*64]
CB = O_SPLIT + 1408

LAST_RESULTS = None       # BassKernelResults of the most recent run (for test.py)


def build_module(alpha_val=ALPHA):
    nc = bacc.Bacc("TRN2", target_bir_lowering=False)

    constb_d = nc.dram_tensor("CONSTB", [128, CB], BF16, kind="ExternalInput")
    v2p_d = nc.dram_tensor("V2P", [NG, 128, KG * 4 * 128], BF16, kind="ExternalInput")
    pebc_d = nc.dram_tensor("PEBC", [NG, 128, KG * BP], BF16, kind="ExternalInput")
    v3_d = nc.dram_tensor("V3", [NQ, KC, 128, 2 * D], BF16, kind="ExternalInput")
    out_d = nc.dram_tensor("out", [128, NBT * DT], F32, kind="ExternalOutput")

    with TileContext(nc) as tc:
        with (
            tc.tile_pool(name="const", bufs=1) as cp,
            tc.tile_pool(name="v2s", bufs=3) as v2p_pool,
            tc.tile_pool(name="pbs", bufs=3) as pb_pool,
            tc.tile_pool(name="zs", bufs=6) as z_pool,
            tc.tile_pool(name="v3s", bufs=6) as v3_pool,
            tc.tile_pool(name="s3s", bufs=22) as s_pool,
            tc.tile_pool(name="hps", bufs=6, space="PSUM") as hp,  # 1-bank tiles
            tc.tile_pool(name="accps", bufs=1, space="PSUM") as accp,
        ):
            # ---- constants ----
            cb_s = cp.tile([128, CB], BF16)
            nc.sync.dma_start(out=cb_s[:, 0:O_CHUNK1], in_=constb_d[:, 0:O_CHUNK1])
            nc.sync.dma_start(out=cb_s[:, O_CHUNK1:O_ROWS],
                              in_=constb_d[:, O_CHUNK1:O_ROWS])
            # rows region: only partition 0 carries data -- DMA just that row
            nc.sync.dma_start(out=cb_s[0:1, O_ROWS:O_SPLIT],
                              in_=constb_d[0:1, O_ROWS:O_SPLIT])
            nc.sync.dma_start(out=cb_s[:, O_SPLIT:CB], in_=constb_d[:, O_SPLIT:CB])

            # ---- group prefetch (issue order on Sync == program order) ----
            v2ts, pbts, v3ts = {}, {}, {}

            def fetch_g(g):
                pbt = pb_pool.tile([128, KG, BP], BF16, tag="pb")
                nc.sync.dma_start(
                    out=pbt[:, :, :],
                    in_=pebc_d[g, :, :].rearrange("p (ki b) -> p ki b", ki=KG))
                v2t = v2p_pool.tile([128, KG, 2, 2, 128], BF16, tag="v2")
                nc.sync.dma_start(
                    out=v2t[:, :, :, :, :],
                    in_=v2p_d[g, :, :].rearrange(
                        "p (ki dc eh e) -> p ki dc eh e", ki=KG, dc=2, eh=2))
                v2ts[g], pbts[g] = v2t, pbt

            def fetch_v3(q):
                v3t = v3_pool.tile([128, KC, 2 * D], BF16, tag="v3")
                nc.sync.dma_start(
                    out=v3t[:, :, :],
                    in_=v3_d[q, :, :, :].rearrange("kc p de -> p kc de"))
                v3ts[q] = v3t

            fetch_g(0)
            fetch_g(1)
            fetch_v3(0)
            fetch_v3(1)

            # ---- PE warm-up: matmuls on a zero tile while the head DMAs
            # are in flight, so the HAM clock gate reaches 8/8 (2.4 GHz)
            # before the first real matmul issues.
            warm = s_pool.tile([128, 2 * D], BF16, tag="S")
            nc.gpsimd.memset(warm[:, :], 0.0)
            wout = hp.tile([128, BP], F32, tag="H")
            for i in range(9):
                nc.tensor.matmul(wout[:, :], warm[:, 0:128], warm[:, :],
                                 start=True, stop=True)

            def petkc(ci, kc):
                o = O_PETKC + (ci * 2 + kc) * BP
                return cb_s[:, o:o + BP]

            x1T_s = cp.tile([128, KC, BP], BF16)
            x2pT_s = cp.tile([128, KC, BP], BF16)
            x2p_s = cp.tile([128, NBT, D], F32)
            out_s = cp.tile([128, NBT, DT], F32)

            def petk(kc):
                o = O_PETK + kc * BP
                return cb_s[:, o:o + BP]

            def petkb(kc, bt):
                o = O_PETK + kc * BP + bt * 128
                return cb_s[:, o:o + 128]

            def st4(base, i, j):
                o = base + (i * 2 + j) * 128
                return cb_s[:, o:o + 128]

            ident = cb_s[:, O_ID:O_ID + 128]
            ones_row = cb_s[0:1, O_R_ONES:O_R_ONES + BP]

            # ================= Layer 1 (transposed) =================
            # x1T = prelu(c0*h1aT + c1*h1bT + bb1T) with the coods scaling
            # folded into host-prescaled moving operands (petkC): one PSUM
            # accumulation per dch, no vector-engine combine at all.
            h1 = [hp.tile([128, BP], F32, tag="H", name=f"h1{i}") for i in range(2)]
            for dch in range(2):
                nc.tensor.matmul(h1[dch][:, :], st4(O_W1A, dch, 0), petkc(0, 0),
                                 start=True, stop=False)
                nc.tensor.matmul(h1[dch][:, :], st4(O_W1A, dch, 1), petkc(0, 1),
                                 start=False, stop=False)
            for dch in range(2):
                nc.tensor.matmul(h1[dch][:, :], st4(O_W1B, dch, 0), petkc(1, 0),
                                 start=False, stop=False)
                nc.tensor.matmul(h1[dch][:, :], st4(O_W1B, dch, 1), petkc(1, 1),
                                 start=False, stop=False)
                nc.tensor.matmul(h1[dch][:, :], st4(O_W1BB, dch, 0), petk(0),
                                 start=False, stop=False)
                nc.tensor.matmul(h1[dch][:, :], st4(O_W1BB, dch, 1), petk(1),
                                 start=False, stop=False)
            for dch in range(2):
                nc.tensor.matmul(h1[dch][:, :],
                                 cb_s[0:1, O_R_B1W + dch * 128:O_R_B1W + (dch + 1) * 128],
                                 cb_s[0:1, O_R_C0:O_R_C0 + BP],
                                 start=False, stop=False)
                nc.tensor.matmul(h1[dch][:, :],
                                 cb_s[0:1, O_R_B1W + 256 + dch * 128:O_R_B1W + 256 + (dch + 1) * 128],
                                 cb_s[0:1, O_R_C1:O_R_C1 + BP],
                                 start=False, stop=False)
                nc.tensor.matmul(h1[dch][:, :],
                                 cb_s[0:1, O_R_B1B + dch * 128:O_R_B1B + (dch + 1) * 128],
                                 ones_row, start=False, stop=True)

            # ---- layer-2 accumulator + pe-only bias matmuls ----
            x2T = accp.tile([128, 2, BP], F32, tag="acc")
            for eh in range(2):
                nc.tensor.matmul(x2T[:, eh, :], st4(O_W2B, eh, 0), petk(0),
                                 start=True, stop=False)
                nc.tensor.matmul(x2T[:, eh, :], st4(O_W2B, eh, 1), petk(1),
                                 start=False, stop=False)
                nc.tensor.matmul(x2T[:, eh, :],
                                 cb_s[0:1, O_R_B2B + eh * 128:O_R_B2B + (eh + 1) * 128],
                                 ones_row, start=False, stop=False)

            for dch in range(2):
                nc.scalar.activation(x1T_s[:, dch, :], h1[dch][:, :],
                                     mybir.ActivationFunctionType.Prelu, alpha=alpha_val)

            # ---- x1-dependent layer-2 bias term: (x1 @ B2)^T ----
            for eh in range(2):
                for dc in range(2):
                    nc.tensor.matmul(x2T[:, eh, :], st4(O_B2, eh, dc),
                                     x1T_s[:, dc, :], start=False, stop=False)

            # ================= Layer 2 main loop =================
            for g in range(NG):
                if g + 2 < NG:
                    fetch_g(g + 2)
                v2t = v2ts.pop(g)
                pbt = pbts.pop(g)
                for kp in range(KG // 2):
                    z = z_pool.tile([128, 2, 2, BP], BF16, tag="z")
                    nc.vector.tensor_tensor(
                        z[:, :, :, :],
                        x1T_s[:, None, :, :].to_broadcast([128, 2, KC, BP]),
                        pbt[:, 2 * kp:2 * kp + 2, None, :].to_broadcast(
                            [128, 2, KC, BP]),
                        mybir.AluOpType.mult)
                    for kl in range(2):
                        ki = 2 * kp + kl
                        last_k = (g == NG - 1 and ki == KG - 1)
                        for dc in range(2):
                            for eh in range(2):
                                nc.tensor.matmul(
                                    x2T[:, eh, :], v2t[:, ki, dc, eh, :],
                                    z[:, kl, dc, :],
                                    start=False, stop=(last_k and dc == 1))

            # x2pT = prelu(x2T)
            for eh in range(2):
                nc.scalar.activation(x2pT_s[:, eh, :], x2T[:, eh, :],
                                     mybir.ActivationFunctionType.Prelu, alpha=alpha_val)

            # ================= Layer 3 =================
            LAGM = 3
            s3s = {}
            b3seg = cb_s[:, O_B3:O_B3 + 2 * DT]
            x3a = None
            for ii in range(NQ + LAGM):
                if ii < NQ:
                    q = ii
                    if q + 2 < NQ:
                        fetch_v3(q + 2)
                    v3t = v3ts.pop(q)
                    h3s = []
                    for bt in range(NBT):
                        h3 = hp.tile([128, BP], F32, tag="H")
                        h3s.append(h3)
                        nc.tensor.matmul(h3[:, :], petkb(0, bt), v3t[:, 0, :],
                                         start=True, stop=False)
                        nc.tensor.matmul(h3[:, :], petkb(1, bt), v3t[:, 1, :],
                                         start=False, stop=True)
                    if ii == 0:
                        # transpose x2pT -> x2p [b, d] (for layer-3 scaling);
                        # placed after the first h3-gen block so the PE has
                        # work at the L2->L3 boundary while PReLU completes.
                        for eh in range(2):
                            for bt in range(NBT):
                                trt = hp.tile([128, BP], BF16, tag="H")
                                tr = trt[:, 0:128]
                                nc.tensor.transpose(
                                    tr, x2pT_s[:, eh, bt * 128:(bt + 1) * 128],
                                    ident)
                                nc.scalar.activation(
                                    x2p_s[:, bt, eh * 128:(eh + 1) * 128], tr,
                                    mybir.ActivationFunctionType.Copy)
                        x3a = accp.tile([128, NBT, D], F32, tag="acc")
                        for bt in range(NBT):
                            nc.tensor.matmul(x3a[:, bt, 0:DT], petkb(0, bt),
                                             cb_s[:, O_W3B:O_W3B + DT],
                                             start=(bt % 2 == 0), stop=False)
                            nc.tensor.matmul(x3a[:, bt, 0:DT], petkb(1, bt),
                                             cb_s[:, O_W3B + DT:O_W3B + 2 * DT],
                                             start=False, stop=False)
                            nc.tensor.matmul(x3a[:, bt, 0:DT],
                                             ones_row[0:1, bt * 128:(bt + 1) * 128],
                                             cb_s[0:1, O_R_B3B:O_R_B3B + DT],
                                             start=False, stop=False)
                            nc.tensor.matmul(x3a[:, bt, 0:DT],
                                             x2pT_s[:, 0, bt * 128:(bt + 1) * 128],
                                             b3seg[:, 0:DT], start=False, stop=False)
                            nc.tensor.matmul(x3a[:, bt, 0:DT],
                                             x2pT_s[:, 1, bt * 128:(bt + 1) * 128],
                                             b3seg[:, DT:2 * DT], start=False,
                                             stop=False)
                    # scale: s3[b, (dl, t)] = h3[b, (dl, t)] * x2p[b, 8q+dl]
                    for bt in range(NBT):
                        ht = h3s[bt][:, :]
                        s3 = s_pool.tile([128, 2 * D], BF16, tag="S")
                        s3s[(q, bt)] = s3
                        nc.vector.tensor_tensor(
                            s3[:, :].rearrange("p (dl t) -> p dl t", dl=8),
                            ht.rearrange("p (dl t) -> p dl t", dl=8),
                            x2p_s[:, bt, 8 * q:8 * q + 8][:, :, None]
                            .to_broadcast([128, 8, DT]),
                            mybir.AluOpType.mult)
                if ii >= LAGM:
                    q = ii - LAGM
                    for bt in range(NBT):
                        s3 = s3s.pop((q, bt))
                        for half in range(2):
                            nc.tensor.matmul(x3a[:, bt, :], ident,
                                             s3[:, half * D:(half + 1) * D],
                                             start=False,
                                             stop=(q == NQ - 1 and half == 1))

            # combine the 4 column groups: x3[t] = sum_g x3a[(g, t)]
            # via one strided tensor_reduce per bt (innermost axis = g)
            # combine per PSUM-bank pair; ship each half as soon as its
            # reduces land so the first HBM write receipt overlaps the rest
            for bh in range(2):
                for bt in (2 * bh, 2 * bh + 1):
                    nc.vector.tensor_reduce(
                        out_s[:, bt, :],
                        x3a[:, bt, :].rearrange("p (g t) -> p t g", g=4),
                        mybir.AxisListType.X, mybir.AluOpType.add)
                nc.sync.dma_start(out=out_d[:, bh * 2 * DT:(bh + 1) * 2 * DT],
                                  in_=out_s[:, 2 * bh:2 * bh + 2, :])

    nc.compile()
    return nc


def _kc_split(mat):
    """[256, F] -> [128, 2*F] with row p holding [chunk0(p), chunk1(p)]."""
    f = mat.shape[1]
    return np.ascontiguousarray(
        mat.reshape(KC, 128, f).transpose(1, 0, 2).reshape(128, KC * f))


def _prep_host(coods, pe, W1w, b1w, W1b, b1b, W2w, b2w, W2b, b2b, W3w, b3w, W3b, b3b):
    import ml_dtypes
    bf = ml_dtypes.bfloat16
    f = np.float32
    b1w = np.asarray(b1w, f)
    W1w = np.asarray(W1w, f)
    W1b = np.asarray(W1b, f)
    W2w = np.asarray(W2w, f)
    W2b = np.asarray(W2b, f)
    W3w = np.asarray(W3w, f)

    base = np.zeros((128, CB), dtype=f)

    def put4(off, tiles):
        for idx, t in enumerate(tiles):
            base[:, off + idx * 128:off + (idx + 1) * 128] = t

    # h1aT / h1bT stationaries: lhsT[k, d] = W1w[half*D + dch*128 + d, kc*128 + k]
    for half, off in ((0, O_W1A), (1, O_W1B)):
        Wh = W1w[half * D:(half + 1) * D]           # [256 d, 256 k]
        put4(off, [Wh[dch * 128:(dch + 1) * 128, kc * 128:(kc + 1) * 128].T
                   for dch in range(2) for kc in range(2)])
    # bb1T stationaries from W1b
    put4(O_W1BB, [W1b[dch * 128:(dch + 1) * 128, kc * 128:(kc + 1) * 128].T
                  for dch in range(2) for kc in range(2)])
    # bb2T stationaries from W2b: lhsT[k, e] = W2b[eh*128+e, kc*128+k]
    put4(O_W2B, [W2b[eh * 128:(eh + 1) * 128, kc * 128:(kc + 1) * 128].T
                 for eh in range(2) for kc in range(2)])
    # B2 stationaries from b2w: lhsT[d, e] = b2w.reshape(D, D)[dc*128+d, eh*128+e]
    B2full = np.asarray(b2w, f).reshape(D, D)
    put4(O_B2, [B2full[dc * 128:(dc + 1) * 128, eh * 128:(eh + 1) * 128]
                for eh in range(2) for dc in range(2)])
    base[:, O_ID:O_ID + 128] = np.eye(128, dtype=f)
    base[:, O_W3B:O_W3B + 2 * DT] = _kc_split(
        np.ascontiguousarray(np.asarray(W3b, f).T))
    base[:, O_B3:O_B3 + 2 * DT] = _kc_split(np.asarray(b3w, f).reshape(D, DT))
    base[0, O_R_B1B:O_R_B1B + D] = b1b
    base[0, O_R_B2B:O_R_B2B + D] = b2b
    base[0, O_R_B3B:O_R_B3B + DT] = b3b
    base[0, O_R_ONES:O_R_ONES + BP] = 1.0
    base[0, O_R_B1W:O_R_B1W + 2 * D] = b1w

    # V2P[g, p, ki, dc, eh, e] = W2w[(dc*128+p)*D + eh*128+e, g*KG+ki]
    V2P = np.ascontiguousarray(
        W2w.reshape(2, 128, 2, 128, D)          # [dc, p, eh, e, k]
        .transpose(4, 1, 0, 2, 3)               # [k, p, dc, eh, e]
        .reshape(NG, KG, 128, 2, 2, 128)
        .transpose(0, 2, 1, 3, 4, 5)            # [g, p, ki, dc, eh, e]
        .reshape(NG, 128, KG * 4 * 128)).astype(bf)

    # V3 moving layout (identical to v1)
    V3n = np.ascontiguousarray(
        W3w.reshape(D // 4, 4, DT, D).transpose(0, 3, 1, 2).reshape(D // 4, D, 4 * DT))
    V3 = np.ascontiguousarray(
        V3n.reshape(NQ, 2, KC, 128, D).transpose(0, 2, 3, 1, 4)
        .reshape(NQ, KC, 128, 2 * D)).astype(bf)

    in_maps = []
    for i in range(NCORES):
        sl = slice(i * BP, (i + 1) * BP)
        pe_sh = np.asarray(pe[sl], f)               # [BP, D]
        cood_sh = np.asarray(coods[sl], f)          # [BP, 2]
        const = base.copy()

        def kcsp(mat):
            return np.ascontiguousarray(
                mat.T.reshape(KC, 128, BP).transpose(1, 0, 2).reshape(128, KC * BP))

        const[:, O_PETK:O_PETK + KC * BP] = kcsp(pe_sh)
        for ci in range(2):
            const[:, O_PETKC + ci * KC * BP:O_PETKC + (ci + 1) * KC * BP] = kcsp(
                pe_sh * cood_sh[:, ci:ci + 1])
        const[0, O_R_C0:O_R_C0 + BP] = cood_sh[:, 0]
        const[0, O_R_C1:O_R_C1 + BP] = cood_sh[:, 1]
        pebc = np.ascontiguousarray(
            np.broadcast_to(
                pe_sh.T.reshape(NG, KG, BP)[:, None, :, :], (NG, 128, KG, BP))
            .reshape(NG, 128, KG * BP)).astype(bf)
        in_maps.append({"CONSTB": const.astype(bf),
                        "V2P": V2P, "PEBC": pebc, "V3": V3})
    return in_maps


def kernel(coods, pe, W1w, b1w, W1b, b1b, W2w, b2w, W2b, b2b,
           W3w, b3w, W3b, b3b, alpha):
    global LAST_RESULTS
    in_maps = _prep_host(coods, pe, W1w, b1w, W1b, b1b, W2w, b2w,
                         W2b, b2b, W3w, b3w, W3b, b3b)
    nc = build_module(float(np.asarray(alpha).reshape(-1)[0]))
    trace = bool(int(os.environ.get("KERNEL_TRACE", "0")))
    res = run_bass_kernel_spmd(nc, in_maps, core_ids=list(range(NCORES)), trace=trace)
    LAST_RESULTS = res
    parts = []
    for o in res.results:
        oc = o["out"].reshape(128, NBT, DT)
        parts.append(np.ascontiguousarray(oc.transpose(1, 0, 2)).reshape(BP, DT))
    return np.concatenate(parts, axis=0).astype(np.float32)



# revision 23
# speedup vs baseline: 1.0695x; 1.0695x over previous
"""Trainium2 Bass kernel for nn_MetaPN (hypernetwork MLP), v3.

v3 over v2 (~357us -> ~333us min, PE busy ~93%, PE gaps ~2-4us):
  - all layer-3 scale ops on VectorE (the ScalarE per-d path measured ~2x
    its modeled cost and was the L3 bottleneck)
  - CONSTB laid out so one small first DMA chunk unblocks the first 4
    matmuls; v2p/pbt/v3 tiles for the first groups prefetched explicitly
  - PE warm-up matmuls on a memset tile while head DMAs are in flight
    (HAM clock gate reaches 8/8 before real work)
  - layer-3 bias matmuls + x2 transposes interleaved into the first
    h3-generation block to fill the L2->L3 pipeline boundary
  - CONSTB rows region DMAs only partition 0 (was moving 704KB of zeros
    in the head-critical window), W3b/b3w bias segments unpadded (N=64
    bias matmuls), per-bank-pair output DMA, LAGM=3 drain
  Run-to-run variance is +/-8-20% from P0 power-state downclock with all
  8 cores at high PE utilization; compare minima across runs.

v2 notes:

Math (per sample b):
  x1 = prelu(coods @ w1 + bb1),  w1 = (pe @ W1w.T + b1w).reshape(2, D)
  x2 = prelu(sum_d x1[d] * w2[d, :] + bb2),  w2 = (pe @ W2w.T + b2w).reshape(D, D)
  x3 = sum_d x2[d] * w3[d, :] + bb3,         w3 = (pe @ W3w.T + b3w).reshape(D, DT)

v2 strategy (pure data parallel over batch, 8 cores x 512 samples):
  Layer 2 is computed TRANSPOSED with the per-sample bilinear form folded
  into one long PSUM accumulation:
      x2T[e, b] = sum_{k, d} V2[d, e, k] * (x1[b, d] * pe[b, k])
  - moving operand Z_k[d, b] = x1T[d, b] * peBC_k[:, b], where peBC_k is
    pe[:, k] replicated across partitions HOST-SIDE (DMA'd from DRAM);
    Z is built by single DVE tensor_tensor ops (SBUF-only, bf16).
  - stationary operands are host-permuted V2 slices [128 d x 128 e],
    4 matmuls (dc x eh) of N=512 per k accumulate into 2 PSUM banks.
  This removes the entire per-d scaling stage and all identity-matmul
  accumulation for layer 2 (the v1 bottleneck: ScalarE 80% busy).
  Layer 1 is computed directly in transposed form (x1T) with the coods
  combine using host-replicated coods rows.
  Layer 3 keeps weight-gen + scale + identity-accumulate, but the scale
  stage is one broadcast-AP tensor_tensor per (q, bt) on DVE (+ ScalarE
  per-d ops for one bt) instead of 8 tiny ops.
"""

import os

import numpy as np

import concourse.bass as bass
from concourse import bacc
import concourse.mybir as mybir
from concourse.tile import TileContext
from concourse.bass_utils import run_bass_kernel_spmd

D = 256
DT = 64
B = 4096
NCORES = 8
BP = B // NCORES          # samples per core = 512
NBT = BP // 128           # batch tiles per core = 4
KC = 2                    # contraction chunks of 128 over k (=D=256)
KG = 8                    # k's per DMA group for layer 2
NG = D // KG              # 32 k-groups
NQ = D // 8               # layer-3 d-groups (8 d's each)
ALPHA = 0.25              # PReLU alpha (nn.PReLU default from setup_inputs)

F32 = mybir.dt.float32
BF16 = mybir.dt.bfloat16

# CONSTB (bf16) column offsets -- L1-critical columns first so the first
# (small) DMA chunk unblocks the first 4 matmuls as early as possible.
O_W1A = 0                 # h1aT stationaries (dch,kc) 4x128
O_PETKC = 512             # peT*c0 / peT*c1 kc-split [128, 2*2*512]
O_PETK = 2560             # peT kc-split     [128, 2*512]
O_CHUNK1 = 1536           # end of the first DMA chunk (W1A + petkc ci=0)
O_W1B = 3584              # h1bT stationaries (dch,kc) 4x128
O_W1BB = 4096             # bb1T stationaries (dch,kc) 4x128
O_ROWS = 4608             # partition-0 rows
O_R_B1B = O_ROWS
O_R_B1W = O_ROWS + 256    # b1w rows: half-a [256], half-b [256]
O_R_C0 = O_ROWS + 768     # coods[:,0] row [512]
O_R_C1 = O_ROWS + 1280    # coods[:,1] row [512]
O_R_ONES = O_ROWS + 1792  # ones row [512]
O_R_B2B = O_ROWS + 2304
O_R_B3B = O_ROWS + 2560
O_SPLIT = O_ROWS + 2816   # 7424: end of the early-DMA region
O_W2B = O_SPLIT           # bb2T stationaries (eh,kc)  4x128
O_B2 = O_SPLIT + 512      # B2 = b2w.reshape  (eh,dc)  4x128
O_ID = O_SPLIT + 1024     # identity [128, 128]
O_W3B = O_SPLIT + 1152    # W3b.T kc-split [128, 2*64]
O_B3 = O_SPLIT + 1280     # B3 = b3w.reshape(D,DT) kc-split [128, 2*64]
CB = O_SPLIT + 1408

LAST_RESULTS = None       # BassKernelResults of the most recent run (for test.py)


def build_module(alpha_val=ALPHA):
    nc = bacc.Bacc("TRN2", target_bir_lowering=False)

    constb_d = nc.dram_tensor("CONSTB", [128, CB], BF16, kind="ExternalInput")
    v2p_d = nc.dram_tensor("V2P", [NG, 128, KG * 4 * 128], BF16, kind="ExternalInput")
    pebc_d = nc.dram_tensor("PEBC", [NG, 128, KG * BP], BF16, kind="ExternalInput")
    v3_d = nc.dram_tensor("V3", [NQ, KC, 128, 2 * D], BF16, kind="ExternalInput")
    out_d = nc.dram_tensor("out", [128, NBT * DT], F32, kind="ExternalOutput")

    with TileContext(nc) as tc:
        with (
            tc.tile_pool(name="const", bufs=1) as cp,
            tc.tile_pool(name="v2s", bufs=3) as v2p_pool,
            tc.tile_pool(name="pbs", bufs=3) as pb_pool,
            tc.tile_pool(name="zs", bufs=6) as z_pool,
            tc.tile_pool(name="v3s", bufs=6) as v3_pool,
            tc.tile_pool(name="s3s", bufs=22) as s_pool,
            tc.tile_pool(name="hps", bufs=6, space="PSUM") as hp,  # 1-bank tiles
            tc.tile_pool(name="accps", bufs=1, space="PSUM") as accp,
        ):
            # ---- constants ----
            cb_s = cp.tile([128, CB], BF16)
            nc.sync.dma_start(out=cb_s[:, 0:O_CHUNK1], in_=constb_d[:, 0:O_CHUNK1])
            nc.sync.dma_start(out=cb_s[:, O_CHUNK1:O_ROWS],
                              in_=constb_d[:, O_CHUNK1:O_ROWS])
            # rows region: only partition 0 carries data -- DMA just that row
            nc.sync.dma_start(out=cb_s[0:1, O_ROWS:O_SPLIT],
                              in_=constb_d[0:1, O_ROWS:O_SPLIT])
            nc.sync.dma_start(out=cb_s[:, O_SPLIT:CB], in_=constb_d[:, O_SPLIT:CB])

            # ---- group prefetch (issue order on Sync == program order) ----
            v2ts, pbts, v3ts = {}, {}, {}

            def fetch_g(g, eng=None):
                eng = eng or nc.sync
                pbt = pb_pool.tile([128, KG, BP], BF16, tag="pb")
                eng.dma_start(
                    out=pbt[:, :, :],
                    in_=pebc_d[g, :, :].rearrange("p (ki b) -> p ki b", ki=KG))
                v2t = v2p_pool.tile([128, KG, 2, 2, 128], BF16, tag="v2")
                eng.dma_start(
                    out=v2t[:, :, :, :, :],
                    in_=v2p_d[g, :, :].rearrange(
                        "p (ki dc eh e) -> p ki dc eh e", ki=KG, dc=2, eh=2))
                v2ts[g], pbts[g] = v2t, pbt

            def fetch_v3(q, eng=None):
                eng = eng or nc.sync
                v3t = v3_pool.tile([128, KC, 2 * D], BF16, tag="v3")
                eng.dma_start(
                    out=v3t[:, :, :],
                    in_=v3_d[q, :, :, :].rearrange("kc p de -> p kc de"))
                v3ts[q] = v3t

            # g0 is head-urgent (sync ring, competes with CONSTB); g1/v3 are
            # not needed until ~27us+, so they ride the ScalarE HWDGE ring,
            # whose issue path starts ~7us late -- keeping ~3MB out of the
            # contended startup window that gates the first L1 matmul.
            fetch_g(0)
            fetch_g(1, eng=nc.scalar)
            fetch_v3(0, eng=nc.scalar)
            fetch_v3(1, eng=nc.scalar)

            # ---- PE warm-up: matmuls on a zero tile while the head DMAs
            # are in flight, so the HAM clock gate reaches 8/8 (2.4 GHz)
            # before the first real matmul issues.
            warm = s_pool.tile([128, 2 * D], BF16, tag="S")
            nc.gpsimd.memset(warm[:, :], 0.0)
            wout = hp.tile([128, BP], F32, tag="H")
            for i in range(9):
                nc.tensor.matmul(wout[:, :], warm[:, 0:128], warm[:, :],
                                 start=True, stop=True)

            def petkc(ci, kc):
                o = O_PETKC + (ci * 2 + kc) * BP
                return cb_s[:, o:o + BP]

            x1T_s = cp.tile([128, KC, BP], BF16)
            x2pT_s = cp.tile([128, KC, BP], BF16)
            x2p_s = cp.tile([128, NBT, D], F32)
            out_s = cp.tile([128, NBT, DT], F32)

            def petk(kc):
                o = O_PETK + kc * BP
                return cb_s[:, o:o + BP]

            def petkb(kc, bt):
                o = O_PETK + kc * BP + bt * 128
                return cb_s[:, o:o + 128]

            def st4(base, i, j):
                o = base + (i * 2 + j) * 128
                return cb_s[:, o:o + 128]

            ident = cb_s[:, O_ID:O_ID + 128]
            ones_row = cb_s[0:1, O_R_ONES:O_R_ONES + BP]

            # ================= Layer 1 (transposed) =================
            # x1T = prelu(c0*h1aT + c1*h1bT + bb1T) with the coods scaling
            # folded into host-prescaled moving operands (petkC): one PSUM
            # accumulation per dch, no vector-engine combine at all.
            h1 = [hp.tile([128, BP], F32, tag="H", name=f"h1{i}") for i in range(2)]
            for dch in range(2):
                nc.tensor.matmul(h1[dch][:, :], st4(O_W1A, dch, 0), petkc(0, 0),
                                 start=True, stop=False)
                nc.tensor.matmul(h1[dch][:, :], st4(O_W1A, dch, 1), petkc(0, 1),
                                 start=False, stop=False)
            for dch in range(2):
                nc.tensor.matmul(h1[dch][:, :], st4(O_W1B, dch, 0), petkc(1, 0),
                                 start=False, stop=False)
                nc.tensor.matmul(h1[dch][:, :], st4(O_W1B, dch, 1), petkc(1, 1),
                                 start=False, stop=False)
                nc.tensor.matmul(h1[dch][:, :], st4(O_W1BB, dch, 0), petk(0),
                                 start=False, stop=False)
                nc.tensor.matmul(h1[dch][:, :], st4(O_W1BB, dch, 1), petk(1),
                                 start=False, stop=False)
            for dch in range(2):
                nc.tensor.matmul(h1[dch][:, :],
                                 cb_s[0:1, O_R_B1W + dch * 128:O_R_B1W + (dch + 1) * 128],
                                 cb_s[0:1, O_R_C0:O_R_C0 + BP],
                                 start=False, stop=False)
                nc.tensor.matmul(h1[dch][:, :],
                                 cb_s[0:1, O_R_B1W + 256 + dch * 128:O_R_B1W + 256 + (dch + 1) * 128],
                                 cb_s[0:1, O_R_C1:O_R_C1 + BP],
                                 start=False, stop=False)
                nc.tensor.matmul(h1[dch][:, :],
                                 cb_s[0:1, O_R_B1B + dch * 128:O_R_B1B + (dch + 1) * 128],
                                 ones_row, start=False, stop=True)

            # ---- layer-2 accumulator + pe-only bias matmuls ----
            x2T = accp.tile([128, 2, BP], F32, tag="acc")
            for eh in range(2):
                nc.tensor.matmul(x2T[:, eh, :], st4(O_W2B, eh, 0), petk(0),
                                 start=True, stop=False)
                nc.tensor.matmul(x2T[:, eh, :], st4(O_W2B, eh, 1), petk(1),
                                 start=False, stop=False)
                nc.tensor.matmul(x2T[:, eh, :],
                                 cb_s[0:1, O_R_B2B + eh * 128:O_R_B2B + (eh + 1) * 128],
                                 ones_row, start=False, stop=False)

            for dch in range(2):
                nc.scalar.activation(x1T_s[:, dch, :], h1[dch][:, :],
                                     mybir.ActivationFunctionType.Prelu, alpha=alpha_val)

            # ---- x1-dependent layer-2 bias term: (x1 @ B2)^T ----
            for eh in range(2):
                for dc in range(2):
                    nc.tensor.matmul(x2T[:, eh, :], st4(O_B2, eh, dc),
                                     x1T_s[:, dc, :], start=False, stop=False)

            # ================= Layer 2 main loop =================
            for g in range(NG):
                if g + 2 < NG:
                    fetch_g(g + 2)
                v2t = v2ts.pop(g)
                pbt = pbts.pop(g)
                for kp in range(KG // 2):
                    z = z_pool.tile([128, 2, 2, BP], BF16, tag="z")
                    nc.vector.tensor_tensor(
                        z[:, :, :, :],
                        x1T_s[:, None, :, :].to_broadcast([128, 2, KC, BP]),
                        pbt[:, 2 * kp:2 * kp + 2, None, :].to_broadcast(
                            [128, 2, KC, BP]),
                        mybir.AluOpType.mult)
                    for kl in range(2):
                        ki = 2 * kp + kl
                        last_k = (g == NG - 1 and ki == KG - 1)
                        for dc in range(2):
                            for eh in range(2):
                                nc.tensor.matmul(
                                    x2T[:, eh, :], v2t[:, ki, dc, eh, :],
                                    z[:, kl, dc, :],
                                    start=False, stop=(last_k and dc == 1))

            # x2pT = prelu(x2T)
            for eh in range(2):
                nc.scalar.activation(x2pT_s[:, eh, :], x2T[:, eh, :],
                                     mybir.ActivationFunctionType.Prelu, alpha=alpha_val)

            # ================= Layer 3 =================
            LAGM = 3
            s3s = {}
            b3seg = cb_s[:, O_B3:O_B3 + 2 * DT]
            x3a = None
            for ii in range(NQ + LAGM):
                if ii < NQ:
                    q = ii
                    if q + 2 < NQ:
                        fetch_v3(q + 2)
                    v3t = v3ts.pop(q)
                    h3s = []
                    for bt in range(NBT):
                        h3 = hp.tile([128, BP], F32, tag="H")
                        h3s.append(h3)
                        nc.tensor.matmul(h3[:, :], petkb(0, bt), v3t[:, 0, :],
                                         start=True, stop=False)
                        nc.tensor.matmul(h3[:, :], petkb(1, bt), v3t[:, 1, :],
                                         start=False, stop=True)
                    if ii == 0:
                        # transpose x2pT -> x2p [b, d] (for layer-3 scaling);
                        # placed after the first h3-gen block so the PE has
                        # work at the L2->L3 boundary while PReLU completes.
                        for eh in range(2):
                            for bt in range(NBT):
                                trt = hp.tile([128, BP], BF16, tag="H")
                                tr = trt[:, 0:128]
                                nc.tensor.transpose(
                                    tr, x2pT_s[:, eh, bt * 128:(bt + 1) * 128],
                                    ident)
                                nc.scalar.activation(
                                    x2p_s[:, bt, eh * 128:(eh + 1) * 128], tr,
                                    mybir.ActivationFunctionType.Copy)
                        x3a = accp.tile([128, NBT, D], F32, tag="acc")
                        for bt in range(NBT):
                            nc.tensor.matmul(x3a[:, bt, 0:DT], petkb(0, bt),
                                             cb_s[:, O_W3B:O_W3B + DT],
                                             start=(bt % 2 == 0), stop=False)
                            nc.tensor.matmul(x3a[:, bt, 0:DT], petkb(1, bt),
                                             cb_s[:, O_W3B + DT:O_W3B + 2 * DT],
                                             start=False, stop=False)
                            nc.tensor.matmul(x3a[:, bt, 0:DT],
                                             ones_row[0:1, bt * 128:(bt + 1) * 128],
                                             cb_s[0:1, O_R_B3B:O_R_B3B + DT],
                                             start=False, stop=False)
                            nc.tensor.matmul(x3a[:, bt, 0:DT],
                                             x2pT_s[:, 0, bt * 128:(bt + 1) * 128],
                                             b3seg[:, 0:DT], start=False, stop=False)
                            nc.tensor.matmul(x3a[:, bt, 0:DT],
                                             x2pT_s[:, 1, bt * 128:(bt + 1) * 128],
                                             b3seg[:, DT:2 * DT], start=False,
                                             stop=False)
                    # scale: s3[b, (dl, t)] = h3[b, (dl, t)] * x2p[b, 8q+dl]
                    for bt in range(NBT):
                        ht = h3s[bt][:, :]
                        s3 = s_pool.tile([128, 2 * D], BF16, tag="S")
                        s3s[(q, bt)] = s3
                        nc.vector.tensor_tensor(
                            s3[:, :].rearrange("p (dl t) -> p dl t", dl=8),
                            ht.rearrange("p (dl t) -> p dl t", dl=8),
                            x2p_s[:, bt, 8 * q:8 * q + 8][:, :, None]
                            .to_broadcast([128, 8, DT]),
                            mybir.AluOpType.mult)
                if ii >= LAGM:
                    q = ii - LAGM
                    for bt in range(NBT):
                        s3 = s3s.pop((q, bt))
                        for half in range(2):
                            nc.tensor.matmul(x3a[:, bt, :], ident,
                                             s3[:, half * D:(half + 1) * D],
                                             start=False,
                                             stop=(q == NQ - 1 and half == 1))

            # combine the 4 column groups: x3[t] = sum_g x3a[(g, t)]
            # via one strided tensor_reduce per bt (innermost axis = g)
            # combine per PSUM-bank pair; ship each half as soon as its
            # reduces land so the first HBM write receipt overlaps the rest
            for bh in range(2):
                for bt in (2 * bh, 2 * bh + 1):
                    nc.vector.tensor_reduce(
                        out_s[:, bt, :],
                        x3a[:, bt, :].rearrange("p (g t) -> p t g", g=4),
                        mybir.AxisListType.X, mybir.AluOpType.add)
                nc.sync.dma_start(out=out_d[:, bh * 2 * DT:(bh + 1) * 2 * DT],
                                  in_=out_s[:, 2 * bh:2 * bh + 2, :])

    nc.compile()
    return nc


def _kc_split(mat):
    """[256, F] -> [128, 2*F] with row p holding [chunk0(p), chunk1(p)]."""
    f = mat.shape[1]
    return np.ascontiguousarray(
        mat.reshape(KC, 128, f).transpose(1, 0, 2).reshape(128, KC * f))


def _prep_host(coods, pe, W1w, b1w, W1b, b1b, W2w, b2w, W2b, b2b, W3w, b3w, W3b, b3b):
    import ml_dtypes
    bf = ml_dtypes.bfloat16
    f = np.float32
    b1w = np.asarray(b1w, f)
    W1w = np.asarray(W1w, f)
    W1b = np.asarray(W1b, f)
    W2w = np.asarray(W2w, f)
    W2b = np.asarray(W2b, f)
    W3w = np.asarray(W3w, f)

    base = np.zeros((128, CB), dtype=f)

    def put4(off, tiles):
        for idx, t in enumerate(tiles):
            base[:, off + idx * 128:off + (idx + 1) * 128] = t

    # h1aT / h1bT stationaries: lhsT[k, d] = W1w[half*D + dch*128 + d, kc*128 + k]
    for half, off in ((0, O_W1A), (1, O_W1B)):
        Wh = W1w[half * D:(half + 1) * D]           # [256 d, 256 k]
        put4(off, [Wh[dch * 128:(dch + 1) * 128, kc * 128:(kc + 1) * 128].T
                   for dch in range(2) for kc in range(2)])
    # bb1T stationaries from W1b
    put4(O_W1BB, [W1b[dch * 128:(dch + 1) * 128, kc * 128:(kc + 1) * 128].T
                  for dch in range(2) for kc in range(2)])
    # bb2T stationaries from W2b: lhsT[k, e] = W2b[eh*128+e, kc*128+k]
    put4(O_W2B, [W2b[eh * 128:(eh + 1) * 128, kc * 128:(kc + 1) * 128].T
                 for eh in range(2) for kc in range(2)])
    # B2 stationaries from b2w: lhsT[d, e] = b2w.reshape(D, D)[dc*128+d, eh*128+e]
    B2full = np.asarray(b2w, f).reshape(D, D)
    put4(O_B2, [B2full[dc * 128:(dc + 1) * 128, eh * 128:(eh + 1) * 128]
                for eh in range(2) for dc in range(2)])
    base[:, O_ID:O_ID + 128] = np.eye(128, dtype=f)
    base[:, O_W3B:O_W3B + 2 * DT] = _kc_split(
        np.ascontiguousarray(np.asarray(W3b, f).T))
    base[:, O_B3:O_B3 + 2 * DT] = _kc_split(np.asarray(b3w, f).reshape(D, DT))
    base[0, O_R_B1B:O_R_B1B + D] = b1b
    base[0, O_R_B2B:O_R_B2B + D] = b2b
    base[0, O_R_B3B:O_R_B3B + DT] = b3b
    base[0, O_R_ONES:O_R_ONES + BP] = 1.0
    base[0, O_R_B1W:O_R_B1W + 2 * D] = b1w

    # V2P[g, p, ki, dc, eh, e] = W2w[(dc*128+p)*D + eh*128+e, g*KG+ki]
    V2P = np.ascontiguousarray(
        W2w.reshape(2, 128, 2, 128, D)          # [dc, p, eh, e, k]
        .transpose(4, 1, 0, 2, 3)               # [k, p, dc, eh, e]
        .reshape(NG, KG, 128, 2, 2, 128)
        .transpose(0, 2, 1, 3, 4, 5)            # [g, p, ki, dc, eh, e]
        .reshape(NG, 128, KG * 4 * 128)).astype(bf)

    # V3 moving layout (identical to v1)
    V3n = np.ascontiguousarray(
        W3w.reshape(D // 4, 4, DT, D).transpose(0, 3, 1, 2).reshape(D // 4, D, 4 * DT))
    V3 = np.ascontiguousarray(
        V3n.reshape(NQ, 2, KC, 128, D).transpose(0, 2, 3, 1, 4)
        .reshape(NQ, KC, 128, 2 * D)).astype(bf)

    in_maps = []
    for i in range(NCORES):
        sl = slice(i * BP, (i + 1) * BP)
        pe_sh = np.asarray(pe[sl], f)               # [BP, D]
        cood_sh = np.asarray(coods[sl], f)          # [BP, 2]
        const = base.copy()

        def kcsp(mat):
            return np.ascontiguousarray(
                mat.T.reshape(KC, 128, BP).transpose(1, 0, 2).reshape(128, KC * BP))

        const[:, O_PETK:O_PETK + KC * BP] = kcsp(pe_sh)
        for ci in range(2):
            const[:, O_PETKC + ci * KC * BP:O_PETKC + (ci + 1) * KC * BP] = kcsp(
                pe_sh * cood_sh[:, ci:ci + 1])
        const[0, O_R_C0:O_R_C0 + BP] = cood_sh[:, 0]
        const[0, O_R_C1:O_R_C1 + BP] = cood_sh[:, 1]
        pebc = np.ascontiguousarray(
            np.broadcast_to(
                pe_sh.T.reshape(NG, KG, BP)[:, None, :, :], (NG, 128, KG, BP))
            .reshape(NG, 128, KG * BP)).astype(bf)
        in_maps.append({"CONSTB": const.astype(bf),
                        "V2P": V2P, "PEBC": pebc, "V3": V3})
    return in_maps


def kernel(coods, pe, W1w, b1w, W1b, b1b, W2w, b2w, W2b, b2b,
           W3w, b3w, W3b, b3b, alpha):
    global LAST_RESULTS
    in_maps = _prep_host(coods, pe, W1w, b1w, W1b, b1b, W2w, b2w,
                         W2b, b2b, W3w, b3w, W3b, b3b)
    nc = build_module(float(np.asarray(alpha).reshape(-1)[0]))
    trace = bool(int(os.environ.get("KERNEL_TRACE", "0")))
    res = run_bass_kernel_spmd(nc, in_maps, core_ids=list(range(NCORES)), trace=trace)
    LAST_RESULTS = res
    parts = []
    for o in res.results:
        oc = o["out"].reshape(128, NBT, DT)
        parts.append(np.ascontiguousarray(oc.transpose(1, 0, 2)).reshape(BP, DT))
    return np.concatenate(parts, axis=0).astype(np.float32)



# revision 25
# speedup vs baseline: 1.0930x; 1.0220x over previous
"""Trainium2 Bass kernel for nn_MetaPN (hypernetwork MLP), v3.

v3 over v2 (~357us -> ~333us min, PE busy ~93%, PE gaps ~2-4us):
  - all layer-3 scale ops on VectorE (the ScalarE per-d path measured ~2x
    its modeled cost and was the L3 bottleneck)
  - CONSTB laid out so one small first DMA chunk unblocks the first 4
    matmuls; v2p/pbt/v3 tiles for the first groups prefetched explicitly
  - PE warm-up matmuls on a memset tile while head DMAs are in flight
    (HAM clock gate reaches 8/8 before real work)
  - layer-3 bias matmuls + x2 transposes interleaved into the first
    h3-generation block to fill the L2->L3 pipeline boundary
  - CONSTB rows region DMAs only partition 0 (was moving 704KB of zeros
    in the head-critical window), W3b/b3w bias segments unpadded (N=64
    bias matmuls), per-bank-pair output DMA, LAGM=3 drain
  Run-to-run variance is +/-8-20% from P0 power-state downclock with all
  8 cores at high PE utilization; compare minima across runs.

v2 notes:

Math (per sample b):
  x1 = prelu(coods @ w1 + bb1),  w1 = (pe @ W1w.T + b1w).reshape(2, D)
  x2 = prelu(sum_d x1[d] * w2[d, :] + bb2),  w2 = (pe @ W2w.T + b2w).reshape(D, D)
  x3 = sum_d x2[d] * w3[d, :] + bb3,         w3 = (pe @ W3w.T + b3w).reshape(D, DT)

v2 strategy (pure data parallel over batch, 8 cores x 512 samples):
  Layer 2 is computed TRANSPOSED with the per-sample bilinear form folded
  into one long PSUM accumulation:
      x2T[e, b] = sum_{k, d} V2[d, e, k] * (x1[b, d] * pe[b, k])
  - moving operand Z_k[d, b] = x1T[d, b] * peBC_k[:, b], where peBC_k is
    pe[:, k] replicated across partitions HOST-SIDE (DMA'd from DRAM);
    Z is built by single DVE tensor_tensor ops (SBUF-only, bf16).
  - stationary operands are host-permuted V2 slices [128 d x 128 e],
    4 matmuls (dc x eh) of N=512 per k accumulate into 2 PSUM banks.
  This removes the entire per-d scaling stage and all identity-matmul
  accumulation for layer 2 (the v1 bottleneck: ScalarE 80% busy).
  Layer 1 is computed directly in transposed form (x1T) with the coods
  combine using host-replicated coods rows.
  Layer 3 keeps weight-gen + scale + identity-accumulate, but the scale
  stage is one broadcast-AP tensor_tensor per (q, bt) on DVE (+ ScalarE
  per-d ops for one bt) instead of 8 tiny ops.
"""

import os

import numpy as np

import concourse.bass as bass
from concourse import bacc
import concourse.mybir as mybir
from concourse.tile import TileContext
from concourse.bass_utils import run_bass_kernel_spmd

D = 256
DT = 64
B = 4096
NCORES = 8
BP = B // NCORES          # samples per core = 512
NBT = BP // 128           # batch tiles per core = 4
KC = 2                    # contraction chunks of 128 over k (=D=256)
KG = 8                    # k's per DMA group for layer 2
NG = D // KG              # 32 k-groups
NQ = D // 8               # layer-3 d-groups (8 d's each)
ALPHA = 0.25              # PReLU alpha (nn.PReLU default from setup_inputs)

F32 = mybir.dt.float32
BF16 = mybir.dt.bfloat16

# CONSTB (bf16) column offsets -- L1-critical columns first so the first
# (small) DMA chunk unblocks the first 4 matmuls as early as possible.
O_W1A = 0                 # h1aT stationaries (dch,kc) 4x128
O_PETKC = 512             # peT*c0 / peT*c1 kc-split [128, 2*2*512]
O_PETK = 2560             # peT kc-split     [128, 2*512]
O_CHUNK1 = 1536           # end of the first DMA chunk (W1A + petkc ci=0)
O_W1B = 3584              # h1bT stationaries (dch,kc) 4x128
O_W1BB = 4096             # bb1T stationaries (dch,kc) 4x128
O_ROWS = 4608             # partition-0 rows
O_R_B1B = O_ROWS
O_R_B1W = O_ROWS + 256    # b1w rows: half-a [256], half-b [256]
O_R_C0 = O_ROWS + 768     # coods[:,0] row [512]
O_R_C1 = O_ROWS + 1280    # coods[:,1] row [512]
O_R_ONES = O_ROWS + 1792  # ones row [512]
O_R_B2B = O_ROWS + 2304
O_R_B3B = O_ROWS + 2560
O_SPLIT = O_ROWS + 2816   # 7424: end of the early-DMA region
O_W2B = O_SPLIT           # bb2T stationaries (eh,kc)  4x128
O_B2 = O_SPLIT + 512      # B2 = b2w.reshape  (eh,dc)  4x128
O_ID = O_SPLIT + 1024     # identity [128, 128]
O_W3B = O_SPLIT + 1152    # W3b.T kc-split [128, 2*64]
O_B3 = O_SPLIT + 1280     # B3 = b3w.reshape(D,DT) kc-split [128, 2*64]
CB = O_SPLIT + 1408

LAST_RESULTS = None       # BassKernelResults of the most recent run (for test.py)


def build_module(alpha_val=ALPHA):
    nc = bacc.Bacc("TRN2", target_bir_lowering=False)

    constb_d = nc.dram_tensor("CONSTB", [128, CB], BF16, kind="ExternalInput")
    v2p_d = nc.dram_tensor("V2P", [NG, 128, KG * 4 * 128], BF16, kind="ExternalInput")
    pebc_d = nc.dram_tensor("PEBC", [NG, 128, KG * BP], BF16, kind="ExternalInput")
    v3_d = nc.dram_tensor("V3", [NQ, KC, 128, 2 * D], BF16, kind="ExternalInput")
    out_d = nc.dram_tensor("out", [128, NBT * DT], F32, kind="ExternalOutput")

    with TileContext(nc) as tc:
        with (
            tc.tile_pool(name="const", bufs=1) as cp,
            tc.tile_pool(name="v2s", bufs=3) as v2p_pool,
            tc.tile_pool(name="pbs", bufs=3) as pb_pool,
            tc.tile_pool(name="zs", bufs=6) as z_pool,
            tc.tile_pool(name="v3s", bufs=6) as v3_pool,
            tc.tile_pool(name="s3s", bufs=22) as s_pool,
            tc.tile_pool(name="hps", bufs=6, space="PSUM") as hp,  # 1-bank tiles
            tc.tile_pool(name="accps", bufs=1, space="PSUM") as accp,
        ):
            # ---- constants ----
            cb_s = cp.tile([128, CB], BF16)
            nc.sync.dma_start(out=cb_s[:, 0:O_CHUNK1], in_=constb_d[:, 0:O_CHUNK1])
            nc.sync.dma_start(out=cb_s[:, O_CHUNK1:O_ROWS],
                              in_=constb_d[:, O_CHUNK1:O_ROWS])
            # rows region: only partition 0 carries data -- DMA just that row
            nc.sync.dma_start(out=cb_s[0:1, O_ROWS:O_SPLIT],
                              in_=constb_d[0:1, O_ROWS:O_SPLIT])
            nc.sync.dma_start(out=cb_s[:, O_SPLIT:CB], in_=constb_d[:, O_SPLIT:CB])

            # ---- group prefetch (issue order on Sync == program order) ----
            v2ts, pbts, v3ts = {}, {}, {}

            def fetch_g(g, eng=None):
                eng = eng or nc.sync
                pbt = pb_pool.tile([128, KG, BP], BF16, tag="pb")
                eng.dma_start(
                    out=pbt[:, :, :],
                    in_=pebc_d[g, :, :].rearrange("p (ki b) -> p ki b", ki=KG))
                v2t = v2p_pool.tile([128, KG, 2, 2, 128], BF16, tag="v2")
                eng.dma_start(
                    out=v2t[:, :, :, :, :],
                    in_=v2p_d[g, :, :].rearrange(
                        "p (ki dc eh e) -> p ki dc eh e", ki=KG, dc=2, eh=2))
                v2ts[g], pbts[g] = v2t, pbt

            def fetch_v3(q, eng=None):
                eng = eng or nc.sync
                v3t = v3_pool.tile([128, KC, 2 * D], BF16, tag="v3")
                eng.dma_start(
                    out=v3t[:, :, :],
                    in_=v3_d[q, :, :, :].rearrange("kc p de -> p kc de"))
                v3ts[q] = v3t

            # Only L1/L2-urgent data in the head burst: the startup window is
            # HBM-contended across all 8 cores, and everything issued here
            # round-robins against the chunk that gates the first matmul.
            # v3 tiles are needed ~200us later, so they are primed after the
            # L2 loop's DMAs instead (still lands mid-L2).
            fetch_g(0)
            fetch_g(1)

            # ---- PE warm-up: matmuls on a zero tile while the head DMAs
            # are in flight, so the HAM clock gate reaches 8/8 (2.4 GHz)
            # before the first real matmul issues.
            warm = s_pool.tile([128, 2 * D], BF16, tag="S")
            nc.gpsimd.memset(warm[:, :], 0.0)
            wout = hp.tile([128, BP], F32, tag="H")
            for i in range(9):
                nc.tensor.matmul(wout[:, :], warm[:, 0:128], warm[:, :],
                                 start=True, stop=True)

            def petkc(ci, kc):
                o = O_PETKC + (ci * 2 + kc) * BP
                return cb_s[:, o:o + BP]

            x1T_s = cp.tile([128, KC, BP], BF16)
            x2pT_s = cp.tile([128, KC, BP], BF16)
            x2p_s = cp.tile([128, NBT, D], F32)
            out_s = cp.tile([128, NBT, DT], F32)

            def petk(kc):
                o = O_PETK + kc * BP
                return cb_s[:, o:o + BP]

            def petkb(kc, bt):
                o = O_PETK + kc * BP + bt * 128
                return cb_s[:, o:o + 128]

            def st4(base, i, j):
                o = base + (i * 2 + j) * 128
                return cb_s[:, o:o + 128]

            ident = cb_s[:, O_ID:O_ID + 128]
            ones_row = cb_s[0:1, O_R_ONES:O_R_ONES + BP]

            # ================= Layer 1 (transposed) =================
            # x1T = prelu(c0*h1aT + c1*h1bT + bb1T) with the coods scaling
            # folded into host-prescaled moving operands (petkC): one PSUM
            # accumulation per dch, no vector-engine combine at all.
            h1 = [hp.tile([128, BP], F32, tag="H", name=f"h1{i}") for i in range(2)]
            for dch in range(2):
                nc.tensor.matmul(h1[dch][:, :], st4(O_W1A, dch, 0), petkc(0, 0),
                                 start=True, stop=False)
                nc.tensor.matmul(h1[dch][:, :], st4(O_W1A, dch, 1), petkc(0, 1),
                                 start=False, stop=False)
            for dch in range(2):
                nc.tensor.matmul(h1[dch][:, :], st4(O_W1B, dch, 0), petkc(1, 0),
                                 start=False, stop=False)
                nc.tensor.matmul(h1[dch][:, :], st4(O_W1B, dch, 1), petkc(1, 1),
                                 start=False, stop=False)
                nc.tensor.matmul(h1[dch][:, :], st4(O_W1BB, dch, 0), petk(0),
                                 start=False, stop=False)
                nc.tensor.matmul(h1[dch][:, :], st4(O_W1BB, dch, 1), petk(1),
                                 start=False, stop=False)
            for dch in range(2):
                nc.tensor.matmul(h1[dch][:, :],
                                 cb_s[0:1, O_R_B1W + dch * 128:O_R_B1W + (dch + 1) * 128],
                                 cb_s[0:1, O_R_C0:O_R_C0 + BP],
                                 start=False, stop=False)
                nc.tensor.matmul(h1[dch][:, :],
                                 cb_s[0:1, O_R_B1W + 256 + dch * 128:O_R_B1W + 256 + (dch + 1) * 128],
                                 cb_s[0:1, O_R_C1:O_R_C1 + BP],
                                 start=False, stop=False)
                nc.tensor.matmul(h1[dch][:, :],
                                 cb_s[0:1, O_R_B1B + dch * 128:O_R_B1B + (dch + 1) * 128],
                                 ones_row, start=False, stop=True)

            # ---- layer-2 accumulator + pe-only bias matmuls ----
            x2T = accp.tile([128, 2, BP], F32, tag="acc")
            for eh in range(2):
                nc.tensor.matmul(x2T[:, eh, :], st4(O_W2B, eh, 0), petk(0),
                                 start=True, stop=False)
                nc.tensor.matmul(x2T[:, eh, :], st4(O_W2B, eh, 1), petk(1),
                                 start=False, stop=False)
                nc.tensor.matmul(x2T[:, eh, :],
                                 cb_s[0:1, O_R_B2B + eh * 128:O_R_B2B + (eh + 1) * 128],
                                 ones_row, start=False, stop=False)

            for dch in range(2):
                nc.scalar.activation(x1T_s[:, dch, :], h1[dch][:, :],
                                     mybir.ActivationFunctionType.Prelu, alpha=alpha_val)

            # ---- x1-dependent layer-2 bias term: (x1 @ B2)^T ----
            for eh in range(2):
                for dc in range(2):
                    nc.tensor.matmul(x2T[:, eh, :], st4(O_B2, eh, dc),
                                     x1T_s[:, dc, :], start=False, stop=False)

            # ================= Layer 2 main loop =================
            for g in range(NG):
                if g + 2 < NG:
                    fetch_g(g + 2)
                v2t = v2ts.pop(g)
                pbt = pbts.pop(g)
                for kp in range(KG // 2):
                    z = z_pool.tile([128, 2, 2, BP], BF16, tag="z")
                    nc.vector.tensor_tensor(
                        z[:, :, :, :],
                        x1T_s[:, None, :, :].to_broadcast([128, 2, KC, BP]),
                        pbt[:, 2 * kp:2 * kp + 2, None, :].to_broadcast(
                            [128, 2, KC, BP]),
                        mybir.AluOpType.mult)
                    for kl in range(2):
                        ki = 2 * kp + kl
                        last_k = (g == NG - 1 and ki == KG - 1)
                        for dc in range(2):
                            for eh in range(2):
                                nc.tensor.matmul(
                                    x2T[:, eh, :], v2t[:, ki, dc, eh, :],
                                    z[:, kl, dc, :],
                                    start=False, stop=(last_k and dc == 1))

            # x2pT = prelu(x2T)
            for eh in range(2):
                nc.scalar.activation(x2pT_s[:, eh, :], x2T[:, eh, :],
                                     mybir.ActivationFunctionType.Prelu, alpha=alpha_val)

            # ================= Layer 3 =================
            fetch_v3(0)
            fetch_v3(1)
            LAGM = 3
            s3s = {}
            b3seg = cb_s[:, O_B3:O_B3 + 2 * DT]
            x3a = None
            for ii in range(NQ + LAGM):
                if ii < NQ:
                    q = ii
                    if q + 2 < NQ:
                        fetch_v3(q + 2)
                    v3t = v3ts.pop(q)
                    h3s = []
                    for bt in range(NBT):
                        h3 = hp.tile([128, BP], F32, tag="H")
                        h3s.append(h3)
                        nc.tensor.matmul(h3[:, :], petkb(0, bt), v3t[:, 0, :],
                                         start=True, stop=False)
                        nc.tensor.matmul(h3[:, :], petkb(1, bt), v3t[:, 1, :],
                                         start=False, stop=True)
                    if ii == 0:
                        # transpose x2pT -> x2p [b, d] (for layer-3 scaling);
                        # placed after the first h3-gen block so the PE has
                        # work at the L2->L3 boundary while PReLU completes.
                        for eh in range(2):
                            for bt in range(NBT):
                                trt = hp.tile([128, BP], BF16, tag="H")
                                tr = trt[:, 0:128]
                                nc.tensor.transpose(
                                    tr, x2pT_s[:, eh, bt * 128:(bt + 1) * 128],
                                    ident)
                                nc.scalar.activation(
                                    x2p_s[:, bt, eh * 128:(eh + 1) * 128], tr,
                                    mybir.ActivationFunctionType.Copy)
                        x3a = accp.tile([128, NBT, D], F32, tag="acc")
                        for bt in range(NBT):
                            nc.tensor.matmul(x3a[:, bt, 0:DT], petkb(0, bt),
                                             cb_s[:, O_W3B:O_W3B + DT],
                                             start=(bt % 2 == 0), stop=False)
                            nc.tensor.matmul(x3a[:, bt, 0:DT], petkb(1, bt),
                                             cb_s[:, O_W3B + DT:O_W3B + 2 * DT],
                                             start=False, stop=False)
                            nc.tensor.matmul(x3a[:, bt, 0:DT],
                                             ones_row[0:1, bt * 128:(bt + 1) * 128],
                                             cb_s[0:1, O_R_B3B:O_R_B3B + DT],
                                             start=False, stop=False)
                            nc.tensor.matmul(x3a[:, bt, 0:DT],
                                             x2pT_s[:, 0, bt * 128:(bt + 1) * 128],
                                             b3seg[:, 0:DT], start=False, stop=False)
                            nc.tensor.matmul(x3a[:, bt, 0:DT],
                                             x2pT_s[:, 1, bt * 128:(bt + 1) * 128],
                                             b3seg[:, DT:2 * DT], start=False,
                                             stop=False)
                    # scale: s3[b, (dl, t)] = h3[b, (dl, t)] * x2p[b, 8q+dl]
                    for bt in range(NBT):
                        ht = h3s[bt][:, :]
                        s3 = s_pool.tile([128, 2 * D], BF16, tag="S")
                        s3s[(q, bt)] = s3
                        nc.vector.tensor_tensor(
                            s3[:, :].rearrange("p (dl t) -> p dl t", dl=8),
                            ht.rearrange("p (dl t) -> p dl t", dl=8),
                            x2p_s[:, bt, 8 * q:8 * q + 8][:, :, None]
                            .to_broadcast([128, 8, DT]),
                            mybir.AluOpType.mult)
                if ii >= LAGM:
                    q = ii - LAGM
                    for bt in range(NBT):
                        s3 = s3s.pop((q, bt))
                        for half in range(2):
                            nc.tensor.matmul(x3a[:, bt, :], ident,
                                             s3[:, half * D:(half + 1) * D],
                                             start=False,
                                             stop=(q == NQ - 1 and half == 1))

            # combine the 4 column groups: x3[t] = sum_g x3a[(g, t)]
            # via one strided tensor_reduce per bt (innermost axis = g)
            # combine per PSUM-bank pair; ship each half as soon as its
            # reduces land so the first HBM write receipt overlaps the rest
            for bh in range(2):
                for bt in (2 * bh, 2 * bh + 1):
                    nc.vector.tensor_reduce(
                        out_s[:, bt, :],
                        x3a[:, bt, :].rearrange("p (g t) -> p t g", g=4),
                        mybir.AxisListType.X, mybir.AluOpType.add)
                nc.sync.dma_start(out=out_d[:, bh * 2 * DT:(bh + 1) * 2 * DT],
                                  in_=out_s[:, 2 * bh:2 * bh + 2, :])

    nc.compile()
    return nc


def _kc_split(mat):
    """[256, F] -> [128, 2*F] with row p holding [chunk0(p), chunk1(p)]."""
    f = mat.shape[1]
    return np.ascontiguousarray(
        mat.reshape(KC, 128, f).transpose(1, 0, 2).reshape(128, KC * f))


def _prep_host(coods, pe, W1w, b1w, W1b, b1b, W2w, b2w, W2b, b2b, W3w, b3w, W3b, b3b):
    import ml_dtypes
    bf = ml_dtypes.bfloat16
    f = np.float32
    b1w = np.asarray(b1w, f)
    W1w = np.asarray(W1w, f)
    W1b = np.asarray(W1b, f)
    W2w = np.asarray(W2w, f)
    W2b = np.asarray(W2b, f)
    W3w = np.asarray(W3w, f)

    base = np.zeros((128, CB), dtype=f)

    def put4(off, tiles):
        for idx, t in enumerate(tiles):
            base[:, off + idx * 128:off + (idx + 1) * 128] = t

    # h1aT / h1bT stationaries: lhsT[k, d] = W1w[half*D + dch*128 + d, kc*128 + k]
    for half, off in ((0, O_W1A), (1, O_W1B)):
        Wh = W1w[half * D:(half + 1) * D]           # [256 d, 256 k]
        put4(off, [Wh[dch * 128:(dch + 1) * 128, kc * 128:(kc + 1) * 128].T
                   for dch in range(2) for kc in range(2)])
    # bb1T stationaries from W1b
    put4(O_W1BB, [W1b[dch * 128:(dch + 1) * 128, kc * 128:(kc + 1) * 128].T
                  for dch in range(2) for kc in range(2)])
    # bb2T stationaries from W2b: lhsT[k, e] = W2b[eh*128+e, kc*128+k]
    put4(O_W2B, [W2b[eh * 128:(eh + 1) * 128, kc * 128:(kc + 1) * 128].T
                 for eh in range(2) for kc in range(2)])
    # B2 stationaries from b2w: lhsT[d, e] = b2w.reshape(D, D)[dc*128+d, eh*128+e]
    B2full = np.asarray(b2w, f).reshape(D, D)
    put4(O_B2, [B2full[dc * 128:(dc + 1) * 128, eh * 128:(eh + 1) * 128]
                for eh in range(2) for dc in range(2)])
    base[:, O_ID:O_ID + 128] = np.eye(128, dtype=f)
    base[:, O_W3B:O_W3B + 2 * DT] = _kc_split(
        np.ascontiguousarray(np.asarray(W3b, f).T))
    base[:, O_B3:O_B3 + 2 * DT] = _kc_split(np.asarray(b3w, f).reshape(D, DT))
    base[0, O_R_B1B:O_R_B1B + D] = b1b
    base[0, O_R_B2B:O_R_B2B + D] = b2b
    base[0, O_R_B3B:O_R_B3B + DT] = b3b
    base[0, O_R_ONES:O_R_ONES + BP] = 1.0
    base[0, O_R_B1W:O_R_B1W + 2 * D] = b1w

    # V2P[g, p, ki, dc, eh, e] = W2w[(dc*128+p)*D + eh*128+e, g*KG+ki]
    V2P = np.ascontiguousarray(
        W2w.reshape(2, 128, 2, 128, D)          # [dc, p, eh, e, k]
        .transpose(4, 1, 0, 2, 3)               # [k, p, dc, eh, e]
        .reshape(NG, KG, 128, 2, 2, 128)
        .transpose(0, 2, 1, 3, 4, 5)            # [g, p, ki, dc, eh, e]
        .reshape(NG, 128, KG * 4 * 128)).astype(bf)

    # V3 moving layout (identical to v1)
    V3n = np.ascontiguousarray(
        W3w.reshape(D // 4, 4, DT, D).transpose(0, 3, 1, 2).reshape(D // 4, D, 4 * DT))
    V3 = np.ascontiguousarray(
        V3n.reshape(NQ, 2, KC, 128, D).transpose(0, 2, 3, 1, 4)
        .reshape(NQ, KC, 128, 2 * D)).astype(bf)

    in_maps = []
    for i in range(NCORES):
        sl = slice(i * BP, (i + 1) * BP)
        pe_sh = np.asarray(pe[sl], f)               # [BP, D]
        cood_sh = np.asarray(coods[sl], f)          # [BP, 2]
        const = base.copy()

        def kcsp(mat):
            return np.ascontiguousarray(
                mat.T.reshape(KC, 128, BP).transpose(1, 0, 2).reshape(128, KC * BP))

        const[:, O_PETK:O_PETK + KC * BP] = kcsp(pe_sh)
        for ci in range(2):
            const[:, O_PETKC + ci * KC * BP:O_PETKC + (ci + 1) * KC * BP] = kcsp(
                pe_sh * cood_sh[:, ci:ci + 1])
        const[0, O_R_C0:O_R_C0 + BP] = cood_sh[:, 0]
        const[0, O_R_C1:O_R_C1 + BP] = cood_sh[:, 1]
        pebc = np.ascontiguousarray(
            np.broadcast_to(
                pe_sh.T.reshape(NG, KG, BP)[:, None, :, :], (NG, 128, KG, BP))
            .reshape(NG, 128, KG * BP)).astype(bf)
        in_maps.append({"CONSTB": const.astype(bf),
                        "V2P": V2P, "PEBC": pebc, "V3": V3})
    return in_maps


def kernel(coods, pe, W1w, b1w, W1b, b1b, W2w, b2w, W2b, b2b,
           W3w, b3w, W3b, b3b, alpha):
    global LAST_RESULTS
    in_maps = _prep_host(coods, pe, W1w, b1w, W1b, b1b, W2w, b2w,
                         W2b, b2b, W3w, b3w, W3b, b3b)
    nc = build_module(float(np.asarray(alpha).reshape(-1)[0]))
    trace = bool(int(os.environ.get("KERNEL_TRACE", "0")))
    res = run_bass_kernel_spmd(nc, in_maps, core_ids=list(range(NCORES)), trace=trace)
    LAST_RESULTS = res
    parts = []
    for o in res.results:
        oc = o["out"].reshape(128, NBT, DT)
        parts.append(np.ascontiguousarray(oc.transpose(1, 0, 2)).reshape(BP, DT))
    return np.concatenate(parts, axis=0).astype(np.float32)



# revision 28
# speedup vs baseline: 1.0944x; 1.0013x over previous
"""Trainium2 Bass kernel for nn_MetaPN (hypernetwork MLP), v3.

v3 over v2 (~357us -> ~333us min, PE busy ~93%, PE gaps ~2-4us):
  - all layer-3 scale ops on VectorE (the ScalarE per-d path measured ~2x
    its modeled cost and was the L3 bottleneck)
  - CONSTB laid out so one small first DMA chunk unblocks the first 4
    matmuls; v2p/pbt/v3 tiles for the first groups prefetched explicitly
  - PE warm-up matmuls on a memset tile while head DMAs are in flight
    (HAM clock gate reaches 8/8 before real work)
  - layer-3 bias matmuls + x2 transposes interleaved into the first
    h3-generation block to fill the L2->L3 pipeline boundary
  - CONSTB rows region DMAs only partition 0 (was moving 704KB of zeros
    in the head-critical window), W3b/b3w bias segments unpadded (N=64
    bias matmuls), per-bank-pair output DMA, LAGM=3 drain
  Run-to-run variance is +/-8-20% from P0 power-state downclock with all
  8 cores at high PE utilization; compare minima across runs.

v2 notes:

Math (per sample b):
  x1 = prelu(coods @ w1 + bb1),  w1 = (pe @ W1w.T + b1w).reshape(2, D)
  x2 = prelu(sum_d x1[d] * w2[d, :] + bb2),  w2 = (pe @ W2w.T + b2w).reshape(D, D)
  x3 = sum_d x2[d] * w3[d, :] + bb3,         w3 = (pe @ W3w.T + b3w).reshape(D, DT)

v2 strategy (pure data parallel over batch, 8 cores x 512 samples):
  Layer 2 is computed TRANSPOSED with the per-sample bilinear form folded
  into one long PSUM accumulation:
      x2T[e, b] = sum_{k, d} V2[d, e, k] * (x1[b, d] * pe[b, k])
  - moving operand Z_k[d, b] = x1T[d, b] * peBC_k[:, b], where peBC_k is
    pe[:, k] replicated across partitions HOST-SIDE (DMA'd from DRAM);
    Z is built by single DVE tensor_tensor ops (SBUF-only, bf16).
  - stationary operands are host-permuted V2 slices [128 d x 128 e],
    4 matmuls (dc x eh) of N=512 per k accumulate into 2 PSUM banks.
  This removes the entire per-d scaling stage and all identity-matmul
  accumulation for layer 2 (the v1 bottleneck: ScalarE 80% busy).
  Layer 1 is computed directly in transposed form (x1T) with the coods
  combine using host-replicated coods rows.
  Layer 3 keeps weight-gen + scale + identity-accumulate, but the scale
  stage is one broadcast-AP tensor_tensor per (q, bt) on DVE (+ ScalarE
  per-d ops for one bt) instead of 8 tiny ops.
"""

import os

import numpy as np

import concourse.bass as bass
from concourse import bacc
import concourse.mybir as mybir
from concourse.tile import TileContext
from concourse.bass_utils import run_bass_kernel_spmd

D = 256
DT = 64
B = 4096
NCORES = 8
BP = B // NCORES          # samples per core = 512
NBT = BP // 128           # batch tiles per core = 4
KC = 2                    # contraction chunks of 128 over k (=D=256)
KG = 4                    # k's per DMA group for layer 2 (small first group
                          # -> earlier L2 start; supply held by deeper pools)
NG = D // KG              # k-groups
NQ = D // 8               # layer-3 d-groups (8 d's each)
ALPHA = 0.25              # PReLU alpha (nn.PReLU default from setup_inputs)

F32 = mybir.dt.float32
BF16 = mybir.dt.bfloat16

# CONSTB (bf16) column offsets -- L1-critical columns first so the first
# (small) DMA chunk unblocks the first 4 matmuls as early as possible.
O_W1A = 0                 # h1aT stationaries (dch,kc) 4x128
O_PETKC = 512             # peT*c0 / peT*c1 kc-split [128, 2*2*512]
O_PETK = 2560             # peT kc-split     [128, 2*512]
O_CHUNK1 = 1536           # end of the first DMA chunk (W1A + petkc ci=0)
O_W1B = 3584              # h1bT stationaries (dch,kc) 4x128
O_W1BB = 4096             # bb1T stationaries (dch,kc) 4x128
O_ROWS = 4608             # partition-0 rows
O_R_B1B = O_ROWS
O_R_B1W = O_ROWS + 256    # b1w rows: half-a [256], half-b [256]
O_R_C0 = O_ROWS + 768     # coods[:,0] row [512]
O_R_C1 = O_ROWS + 1280    # coods[:,1] row [512]
O_R_ONES = O_ROWS + 1792  # ones row [512]
O_R_B2B = O_ROWS + 2304
O_R_B3B = O_ROWS + 2560
O_SPLIT = O_ROWS + 2816   # 7424: end of the early-DMA region
O_W2B = O_SPLIT           # bb2T stationaries (eh,kc)  4x128
O_B2 = O_SPLIT + 512      # B2 = b2w.reshape  (eh,dc)  4x128
O_ID = O_SPLIT + 1024     # identity [128, 128]
O_W3B = O_SPLIT + 1152    # W3b.T kc-split [128, 2*64]
O_B3 = O_SPLIT + 1280     # B3 = b3w.reshape(D,DT) kc-split [128, 2*64]
CB = O_SPLIT + 1408

LAST_RESULTS = None       # BassKernelResults of the most recent run (for test.py)


def build_module(alpha_val=ALPHA):
    nc = bacc.Bacc("TRN2", target_bir_lowering=False)

    constb_d = nc.dram_tensor("CONSTB", [128, CB], BF16, kind="ExternalInput")
    v2p_d = nc.dram_tensor("V2P", [NG, 128, KG * 4 * 128], BF16, kind="ExternalInput")
    pebc_d = nc.dram_tensor("PEBC", [NG, 128, KG * BP], BF16, kind="ExternalInput")
    v3_d = nc.dram_tensor("V3", [NQ, KC, 128, 2 * D], BF16, kind="ExternalInput")
    out_d = nc.dram_tensor("out", [128, NBT * DT], F32, kind="ExternalOutput")

    with TileContext(nc) as tc:
        with (
            tc.tile_pool(name="const", bufs=1) as cp,
            tc.tile_pool(name="v2s", bufs=5) as v2p_pool,
            tc.tile_pool(name="pbs", bufs=5) as pb_pool,
            tc.tile_pool(name="zs", bufs=6) as z_pool,
            tc.tile_pool(name="v3s", bufs=6) as v3_pool,
            tc.tile_pool(name="s3s", bufs=22) as s_pool,
            tc.tile_pool(name="hps", bufs=6, space="PSUM") as hp,  # 1-bank tiles
            tc.tile_pool(name="accps", bufs=1, space="PSUM") as accp,
        ):
            # ---- constants ----
            cb_s = cp.tile([128, CB], BF16)
            nc.sync.dma_start(out=cb_s[:, 0:O_CHUNK1], in_=constb_d[:, 0:O_CHUNK1])
            nc.sync.dma_start(out=cb_s[:, O_CHUNK1:O_ROWS],
                              in_=constb_d[:, O_CHUNK1:O_ROWS])
            # rows region: only partition 0 carries data -- DMA just that row
            nc.sync.dma_start(out=cb_s[0:1, O_ROWS:O_SPLIT],
                              in_=constb_d[0:1, O_ROWS:O_SPLIT])
            nc.sync.dma_start(out=cb_s[:, O_SPLIT:CB], in_=constb_d[:, O_SPLIT:CB])

            # ---- group prefetch (issue order on Sync == program order) ----
            v2ts, pbts, v3ts = {}, {}, {}

            def fetch_g(g, eng=None):
                eng = eng or nc.sync
                pbt = pb_pool.tile([128, KG, BP], BF16, tag="pb")
                eng.dma_start(
                    out=pbt[:, :, :],
                    in_=pebc_d[g, :, :].rearrange("p (ki b) -> p ki b", ki=KG))
                v2t = v2p_pool.tile([128, KG, 2, 2, 128], BF16, tag="v2")
                eng.dma_start(
                    out=v2t[:, :, :, :, :],
                    in_=v2p_d[g, :, :].rearrange(
                        "p (ki dc eh e) -> p ki dc eh e", ki=KG, dc=2, eh=2))
                v2ts[g], pbts[g] = v2t, pbt

            def fetch_v3(q, eng=None):
                eng = eng or nc.sync
                v3t = v3_pool.tile([128, KC, 2 * D], BF16, tag="v3")
                eng.dma_start(
                    out=v3t[:, :, :],
                    in_=v3_d[q, :, :, :].rearrange("kc p de -> p kc de"))
                v3ts[q] = v3t

            # Only L1/L2-urgent data in the head burst: the startup window is
            # HBM-contended across all 8 cores, and everything issued here
            # round-robins against the chunk that gates the first matmul.
            # v3 tiles are needed ~200us later, so they are primed after the
            # L2 loop's DMAs instead (still lands mid-L2).
            fetch_g(0)
            fetch_g(1)

            # ---- PE warm-up: matmuls on a zero tile while the head DMAs
            # are in flight, so the HAM clock gate reaches 8/8 (2.4 GHz)
            # before the first real matmul issues.
            warm = s_pool.tile([128, 2 * D], BF16, tag="S")
            nc.gpsimd.memset(warm[:, :], 0.0)
            wout = hp.tile([128, BP], F32, tag="H")
            for i in range(9):
                nc.tensor.matmul(wout[:, :], warm[:, 0:128], warm[:, :],
                                 start=True, stop=True)

            def petkc(ci, kc):
                o = O_PETKC + (ci * 2 + kc) * BP
                return cb_s[:, o:o + BP]

            x1T_s = cp.tile([128, KC, BP], BF16)
            x2pT_s = cp.tile([128, KC, BP], BF16)
            x2p_s = cp.tile([128, NBT, D], F32)
            out_s = cp.tile([128, NBT, DT], F32)

            def petk(kc):
                o = O_PETK + kc * BP
                return cb_s[:, o:o + BP]

            def petkb(kc, bt):
                o = O_PETK + kc * BP + bt * 128
                return cb_s[:, o:o + 128]

            def st4(base, i, j):
                o = base + (i * 2 + j) * 128
                return cb_s[:, o:o + 128]

            ident = cb_s[:, O_ID:O_ID + 128]
            ones_row = cb_s[0:1, O_R_ONES:O_R_ONES + BP]

            # ================= Layer 1 (transposed) =================
            # x1T = prelu(c0*h1aT + c1*h1bT + bb1T) with the coods scaling
            # folded into host-prescaled moving operands (petkC): one PSUM
            # accumulation per dch, no vector-engine combine at all.
            h1 = [hp.tile([128, BP], F32, tag="H", name=f"h1{i}") for i in range(2)]
            for dch in range(2):
                nc.tensor.matmul(h1[dch][:, :], st4(O_W1A, dch, 0), petkc(0, 0),
                                 start=True, stop=False)
                nc.tensor.matmul(h1[dch][:, :], st4(O_W1A, dch, 1), petkc(0, 1),
                                 start=False, stop=False)
            for dch in range(2):
                nc.tensor.matmul(h1[dch][:, :], st4(O_W1B, dch, 0), petkc(1, 0),
                                 start=False, stop=False)
                nc.tensor.matmul(h1[dch][:, :], st4(O_W1B, dch, 1), petkc(1, 1),
                                 start=False, stop=False)
                nc.tensor.matmul(h1[dch][:, :], st4(O_W1BB, dch, 0), petk(0),
                                 start=False, stop=False)
                nc.tensor.matmul(h1[dch][:, :], st4(O_W1BB, dch, 1), petk(1),
                                 start=False, stop=False)
            for dch in range(2):
                nc.tensor.matmul(h1[dch][:, :],
                                 cb_s[0:1, O_R_B1W + dch * 128:O_R_B1W + (dch + 1) * 128],
                                 cb_s[0:1, O_R_C0:O_R_C0 + BP],
                                 start=False, stop=False)
                nc.tensor.matmul(h1[dch][:, :],
                                 cb_s[0:1, O_R_B1W + 256 + dch * 128:O_R_B1W + 256 + (dch + 1) * 128],
                                 cb_s[0:1, O_R_C1:O_R_C1 + BP],
                                 start=False, stop=False)
                nc.tensor.matmul(h1[dch][:, :],
                                 cb_s[0:1, O_R_B1B + dch * 128:O_R_B1B + (dch + 1) * 128],
                                 ones_row, start=False, stop=True)

            # ---- layer-2 accumulator + pe-only bias matmuls ----
            x2T = accp.tile([128, 2, BP], F32, tag="acc")
            for eh in range(2):
                nc.tensor.matmul(x2T[:, eh, :], st4(O_W2B, eh, 0), petk(0),
                                 start=True, stop=False)
                nc.tensor.matmul(x2T[:, eh, :], st4(O_W2B, eh, 1), petk(1),
                                 start=False, stop=False)
                nc.tensor.matmul(x2T[:, eh, :],
                                 cb_s[0:1, O_R_B2B + eh * 128:O_R_B2B + (eh + 1) * 128],
                                 ones_row, start=False, stop=False)

            for dch in range(2):
                nc.scalar.activation(x1T_s[:, dch, :], h1[dch][:, :],
                                     mybir.ActivationFunctionType.Prelu, alpha=alpha_val)

            # ---- x1-dependent layer-2 bias term: (x1 @ B2)^T ----
            for eh in range(2):
                for dc in range(2):
                    nc.tensor.matmul(x2T[:, eh, :], st4(O_B2, eh, dc),
                                     x1T_s[:, dc, :], start=False, stop=False)

            # ================= Layer 2 main loop =================
            for g in range(NG):
                for gg in range(g + 2, min(g + 4, NG)):
                    if gg not in v2ts:
                        fetch_g(gg)
                v2t = v2ts.pop(g)
                pbt = pbts.pop(g)
                for kp in range(KG // 2):
                    z = z_pool.tile([128, 2, 2, BP], BF16, tag="z")
                    nc.vector.tensor_tensor(
                        z[:, :, :, :],
                        x1T_s[:, None, :, :].to_broadcast([128, 2, KC, BP]),
                        pbt[:, 2 * kp:2 * kp + 2, None, :].to_broadcast(
                            [128, 2, KC, BP]),
                        mybir.AluOpType.mult)
                    for kl in range(2):
                        ki = 2 * kp + kl
                        last_k = (g == NG - 1 and ki == KG - 1)
                        for dc in range(2):
                            for eh in range(2):
                                nc.tensor.matmul(
                                    x2T[:, eh, :], v2t[:, ki, dc, eh, :],
                                    z[:, kl, dc, :],
                                    start=False, stop=(last_k and dc == 1))

            # x2pT = prelu(x2T)
            for eh in range(2):
                nc.scalar.activation(x2pT_s[:, eh, :], x2T[:, eh, :],
                                     mybir.ActivationFunctionType.Prelu, alpha=alpha_val)

            # ================= Layer 3 =================
            fetch_v3(0)
            fetch_v3(1)
            LAGM = 3
            s3s = {}
            b3seg = cb_s[:, O_B3:O_B3 + 2 * DT]
            x3a = None
            for ii in range(NQ + LAGM):
                if ii < NQ:
                    q = ii
                    if q + 2 < NQ:
                        fetch_v3(q + 2)
                    v3t = v3ts.pop(q)
                    h3s = []
                    for bt in range(NBT):
                        h3 = hp.tile([128, BP], F32, tag="H")
                        h3s.append(h3)
                        nc.tensor.matmul(h3[:, :], petkb(0, bt), v3t[:, 0, :],
                                         start=True, stop=False)
                        nc.tensor.matmul(h3[:, :], petkb(1, bt), v3t[:, 1, :],
                                         start=False, stop=True)
                    if ii == 0:
                        # transpose x2pT -> x2p [b, d] (for layer-3 scaling);
                        # placed after the first h3-gen block so the PE has
                        # work at the L2->L3 boundary while PReLU completes.
                        for eh in range(2):
                            for bt in range(NBT):
                                trt = hp.tile([128, BP], BF16, tag="H")
                                tr = trt[:, 0:128]
                                nc.tensor.transpose(
                                    tr, x2pT_s[:, eh, bt * 128:(bt + 1) * 128],
                                    ident)
                                nc.scalar.activation(
                                    x2p_s[:, bt, eh * 128:(eh + 1) * 128], tr,
                                    mybir.ActivationFunctionType.Copy)
                        x3a = accp.tile([128, NBT, D], F32, tag="acc")
                        for bt in range(NBT):
                            nc.tensor.matmul(x3a[:, bt, 0:DT], petkb(0, bt),
                                             cb_s[:, O_W3B:O_W3B + DT],
                                             start=(bt % 2 == 0), stop=False)
                            nc.tensor.matmul(x3a[:, bt, 0:DT], petkb(1, bt),
                                             cb_s[:, O_W3B + DT:O_W3B + 2 * DT],
                                             start=False, stop=False)
                            nc.tensor.matmul(x3a[:, bt, 0:DT],
                                             ones_row[0:1, bt * 128:(bt + 1) * 128],
                                             cb_s[0:1, O_R_B3B:O_R_B3B + DT],
                                             start=False, stop=False)
                            nc.tensor.matmul(x3a[:, bt, 0:DT],
                                             x2pT_s[:, 0, bt * 128:(bt + 1) * 128],
                                             b3seg[:, 0:DT], start=False, stop=False)
                            nc.tensor.matmul(x3a[:, bt, 0:DT],
                                             x2pT_s[:, 1, bt * 128:(bt + 1) * 128],
                                             b3seg[:, DT:2 * DT], start=False,
                                             stop=False)
                    # scale: s3[b, (dl, t)] = h3[b, (dl, t)] * x2p[b, 8q+dl]
                    for bt in range(NBT):
                        ht = h3s[bt][:, :]
                        s3 = s_pool.tile([128, 2 * D], BF16, tag="S")
                        s3s[(q, bt)] = s3
                        nc.vector.tensor_tensor(
                            s3[:, :].rearrange("p (dl t) -> p dl t", dl=8),
                            ht.rearrange("p (dl t) -> p dl t", dl=8),
                            x2p_s[:, bt, 8 * q:8 * q + 8][:, :, None]
                            .to_broadcast([128, 8, DT]),
                            mybir.AluOpType.mult)
                if ii >= LAGM:
                    q = ii - LAGM
                    for bt in range(NBT):
                        s3 = s3s.pop((q, bt))
                        for half in range(2):
                            nc.tensor.matmul(x3a[:, bt, :], ident,
                                             s3[:, half * D:(half + 1) * D],
                                             start=False,
                                             stop=(q == NQ - 1 and half == 1))

            # combine the 4 column groups: x3[t] = sum_g x3a[(g, t)]
            # via one strided tensor_reduce per bt (innermost axis = g)
            # combine per PSUM-bank pair; ship each half as soon as its
            # reduces land so the first HBM write receipt overlaps the rest
            for bh in range(2):
                for bt in (2 * bh, 2 * bh + 1):
                    nc.vector.tensor_reduce(
                        out_s[:, bt, :],
                        x3a[:, bt, :].rearrange("p (g t) -> p t g", g=4),
                        mybir.AxisListType.X, mybir.AluOpType.add)
                nc.sync.dma_start(out=out_d[:, bh * 2 * DT:(bh + 1) * 2 * DT],
                                  in_=out_s[:, 2 * bh:2 * bh + 2, :])

    nc.compile()
    return nc


def _kc_split(mat):
    """[256, F] -> [128, 2*F] with row p holding [chunk0(p), chunk1(p)]."""
    f = mat.shape[1]
    return np.ascontiguousarray(
        mat.reshape(KC, 128, f).transpose(1, 0, 2).reshape(128, KC * f))


def _prep_host(coods, pe, W1w, b1w, W1b, b1b, W2w, b2w, W2b, b2b, W3w, b3w, W3b, b3b):
    import ml_dtypes
    bf = ml_dtypes.bfloat16
    f = np.float32
    b1w = np.asarray(b1w, f)
    W1w = np.asarray(W1w, f)
    W1b = np.asarray(W1b, f)
    W2w = np.asarray(W2w, f)
    W2b = np.asarray(W2b, f)
    W3w = np.asarray(W3w, f)

    base = np.zeros((128, CB), dtype=f)

    def put4(off, tiles):
        for idx, t in enumerate(tiles):
            base[:, off + idx * 128:off + (idx + 1) * 128] = t

    # h1aT / h1bT stationaries: lhsT[k, d] = W1w[half*D + dch*128 + d, kc*128 + k]
    for half, off in ((0, O_W1A), (1, O_W1B)):
        Wh = W1w[half * D:(half + 1) * D]           # [256 d, 256 k]
        put4(off, [Wh[dch * 128:(dch + 1) * 128, kc * 128:(kc + 1) * 128].T
                   for dch in range(2) for kc in range(2)])
    # bb1T stationaries from W1b
    put4(O_W1BB, [W1b[dch * 128:(dch + 1) * 128, kc * 128:(kc + 1) * 128].T
                  for dch in range(2) for kc in range(2)])
    # bb2T stationaries from W2b: lhsT[k, e] = W2b[eh*128+e, kc*128+k]
    put4(O_W2B, [W2b[eh * 128:(eh + 1) * 128, kc * 128:(kc + 1) * 128].T
                 for eh in range(2) for kc in range(2)])
    # B2 stationaries from b2w: lhsT[d, e] = b2w.reshape(D, D)[dc*128+d, eh*128+e]
    B2full = np.asarray(b2w, f).reshape(D, D)
    put4(O_B2, [B2full[dc * 128:(dc + 1) * 128, eh * 128:(eh + 1) * 128]
                for eh in range(2) for dc in range(2)])
    base[:, O_ID:O_ID + 128] = np.eye(128, dtype=f)
    base[:, O_W3B:O_W3B + 2 * DT] = _kc_split(
        np.ascontiguousarray(np.asarray(W3b, f).T))
    base[:, O_B3:O_B3 + 2 * DT] = _kc_split(np.asarray(b3w, f).reshape(D, DT))
    base[0, O_R_B1B:O_R_B1B + D] = b1b
    base[0, O_R_B2B:O_R_B2B + D] = b2b
    base[0, O_R_B3B:O_R_B3B + DT] = b3b
    base[0, O_R_ONES:O_R_ONES + BP] = 1.0
    base[0, O_R_B1W:O_R_B1W + 2 * D] = b1w

    # V2P[g, p, ki, dc, eh, e] = W2w[(dc*128+p)*D + eh*128+e, g*KG+ki]
    V2P = np.ascontiguousarray(
        W2w.reshape(2, 128, 2, 128, D)          # [dc, p, eh, e, k]
        .transpose(4, 1, 0, 2, 3)               # [k, p, dc, eh, e]
        .reshape(NG, KG, 128, 2, 2, 128)
        .transpose(0, 2, 1, 3, 4, 5)            # [g, p, ki, dc, eh, e]
        .reshape(NG, 128, KG * 4 * 128)).astype(bf)

    # V3 moving layout (identical to v1)
    V3n = np.ascontiguousarray(
        W3w.reshape(D // 4, 4, DT, D).transpose(0, 3, 1, 2).reshape(D // 4, D, 4 * DT))
    V3 = np.ascontiguousarray(
        V3n.reshape(NQ, 2, KC, 128, D).transpose(0, 2, 3, 1, 4)
        .reshape(NQ, KC, 128, 2 * D)).astype(bf)

    in_maps = []
    for i in range(NCORES):
        sl = slice(i * BP, (i + 1) * BP)
        pe_sh = np.asarray(pe[sl], f)               # [BP, D]
        cood_sh = np.asarray(coods[sl], f)          # [BP, 2]
        const = base.copy()

        def kcsp(mat):
            return np.ascontiguousarray(
                mat.T.reshape(KC, 128, BP).transpose(1, 0, 2).reshape(128, KC * BP))

        const[:, O_PETK:O_PETK + KC * BP] = kcsp(pe_sh)
        for ci in range(2):
            const[:, O_PETKC + ci * KC * BP:O_PETKC + (ci + 1) * KC * BP] = kcsp(
                pe_sh * cood_sh[:, ci:ci + 1])
        const[0, O_R_C0:O_R_C0 + BP] = cood_sh[:, 0]
        const[0, O_R_C1:O_R_C1 + BP] = cood_sh[:, 1]
        pebc = np.ascontiguousarray(
            np.broadcast_to(
                pe_sh.T.reshape(NG, KG, BP)[:, None, :, :], (NG, 128, KG, BP))
            .reshape(NG, 128, KG * BP)).astype(bf)
        in_maps.append({"CONSTB": const.astype(bf),
                        "V2P": V2P, "PEBC": pebc, "V3": V3})
    return in_maps


def kernel(coods, pe, W1w, b1w, W1b, b1b, W2w, b2w, W2b, b2b,
           W3w, b3w, W3b, b3b, alpha):
    global LAST_RESULTS
    in_maps = _prep_host(coods, pe, W1w, b1w, W1b, b1b, W2w, b2w,
                         W2b, b2b, W3w, b3w, W3b, b3b)
    nc = build_module(float(np.asarray(alpha).reshape(-1)[0]))
    trace = bool(int(os.environ.get("KERNEL_TRACE", "0")))
    res = run_bass_kernel_spmd(nc, in_maps, core_ids=list(range(NCORES)), trace=trace)
    LAST_RESULTS = res
    parts = []
    for o in res.results:
        oc = o["out"].reshape(128, NBT, DT)
        parts.append(np.ascontiguousarray(oc.transpose(1, 0, 2)).reshape(BP, DT))
    return np.concatenate(parts, axis=0).astype(np.float32)

